# revision 33
# baseline (speedup 1.0000x reference)
"""Trainium2 Bass kernel for a sparse-causal-attention BasicTransformerBlock.

Sharding: pure data-parallel over the 8 video frames (batch=1, f=8) - one
frame per NeuronCore, zero collectives.  Each core receives its own frame
plus frame 0 and the previous frame (the sparse-causal KV sources) and
recomputes LN1 + K/V projections for those locally.

v2 design notes (vs the first working version):
  - All PE operands bf16 (weights pre-cast on host): enables fast-weight-load
    and halves weight DMA.  PSUM accumulation stays fp32.
  - Loops ordered so consecutive matmuls share the stationary operand
    (one LDWEIGHTS per weight tile, streaming 2x512 columns).
  - Attention runs kvt-major over the full 1024-token q range: per (h, kvt)
    one [128kv, 1024q] score tile -> one exp -> AV accumulation.  Softmax
    denominator comes from a ones-column appended to V.
  - Softmax normalization: denominator rows for all 8 heads are collected
    into one [8, 1024] tile, a single reciprocal_approx_fast inverts them,
    and a K=1 PE matmul broadcasts each row across partitions (no DRAM
    round-trips, no single-lane iterative divides).
  - LayerNorm normalize runs on the Vector engine (tensor_scalar) with a
    bf16 output feeding PE transposes; LN scale vectors are folded into the
    consuming projection weights on the host.
"""

import numpy as np

P = 128
S = 1024          # tokens per frame
D = 640
H = 8
DH = 80
KV = 2 * S        # sparse-causal kv tokens (first frame + prev frame)
ENC = 77
ENCP = 80         # padded to 16-byte stride for fp8 DoubleRow
CROSS = 768
FFI = 2560
NQT = S // P      # 8 token tiles
NKT = KV // P     # 16 kv token tiles
ND = D // P       # 5 dim tiles
NE = CROSS // P   # 6 encoder-dim tiles
NM1 = 2 * FFI // P  # 40 ff_w1 out tiles
NK2 = FFI // P      # 20 ff_w2 k tiles
VW = 88           # per-head V stride in vp tiles (80 dh + 1 ones + 7 pad)
SCALE = DH ** -0.5
SW = 16.0         # fp8 weight scale for qkv / cross projections
SW1 = 16.0        # fp8 scale for ff_w1
SW2 = 32.0        # fp8 scale for ff_w2
ESCALE = SCALE / (SW * SW)   # exp scale (descales q and k fp8 weight scales)
ODESC = 1.0 / (SW1 * SW2)    # ffn output descale
EPS = 1e-5
F = 8

_PROGRAM_CACHE = {}


def _build_program(flags):
    import concourse.bass as bass
    import concourse.tile as tile
    from concourse import bacc, mybir
    from concourse.masks import make_identity

    f32 = mybir.dt.float32
    f32r = mybir.dt.float32r
    bf16 = mybir.dt.bfloat16
    f8 = mybir.dt.float8e4
    DR = mybir.MatmulPerfMode.DoubleRow
    AF = mybir.ActivationFunctionType
    Alu = mybir.AluOpType
    PSUM = bass.MemorySpace.PSUM

    (has_qb1, has_kb1, has_vb1, has_ob1, has_q2b, has_ob2, has_fb2) = flags

    nc = bacc.Bacc(None, target_bir_lowering=False)

    hs3_d = nc.dram_tensor("hs3", [3 * S, D], f32, kind="ExternalInput")[:]
    enc_d = nc.dram_tensor("enc", [ENC, CROSS], f32, kind="ExternalInput")[:]
    wq1_d = nc.dram_tensor("wq1", [D, D], f8, kind="ExternalInput")[:]
    wk1_d = nc.dram_tensor("wk1", [D, D], f8, kind="ExternalInput")[:]
    wv1_d = nc.dram_tensor("wv1", [D, D], f8, kind="ExternalInput")[:]
    o1p_d = nc.dram_tensor("o1p", [H, DH, D], bf16, kind="ExternalInput")[:]
    wq2_d = nc.dram_tensor("wq2", [D, D], f8, kind="ExternalInput")[:]
    wk2_d = nc.dram_tensor("wk2", [CROSS, D], f8, kind="ExternalInput")[:]
    wv2_d = nc.dram_tensor("wv2", [CROSS, D], f8, kind="ExternalInput")[:]
    o2p_d = nc.dram_tensor("o2p", [H, DH, D], bf16, kind="ExternalInput")[:]
    w1p_d = nc.dram_tensor("w1p", [NM1, P, ND, P], f8, kind="ExternalInput")[:]
    b1p_d = nc.dram_tensor("b1p", [P, NM1], f32, kind="ExternalInput")[:]
    w2p_d = nc.dram_tensor("w2p", [NK2, P, D], f8, kind="ExternalInput")[:]
    out_d = nc.dram_tensor("out", [S, D], f32, kind="ExternalOutput")[:]

    qb1_d = kb1_d = vb1_d = q2b_d = None
    ob1_d = ob2_d = fb2_d = None
    if has_qb1:
        qb1_d = nc.dram_tensor("qb1", [DH, H], f32, kind="ExternalInput")[:]
    if has_kb1:
        kb1_d = nc.dram_tensor("kb1", [DH, H], f32, kind="ExternalInput")[:]
    if has_vb1:
        vb1_d = nc.dram_tensor("vb1", [DH, H], f32, kind="ExternalInput")[:]
    if has_q2b:
        q2b_d = nc.dram_tensor("q2b", [DH, H], f32, kind="ExternalInput")[:]
    if has_ob1:
        ob1_d = nc.dram_tensor("ob1", [P, D], f32, kind="ExternalInput")[:]
    if has_ob2:
        ob2_d = nc.dram_tensor("ob2", [P, D], f32, kind="ExternalInput")[:]
    if has_fb2:
        fb2_d = nc.dram_tensor("fb2", [P, D], f32, kind="ExternalInput")[:]

    with tile.TileContext(nc) as tc:
        # ---------- whole-kernel pools ----------
        const = tc.alloc_tile_pool(name="const", bufs=1)
        identity = const.tile([P, P], bf16, name="identity")
        make_identity(nc, identity)
        epst = const.tile([P, 1], f32, name="epst")
        nc.vector.memset(epst[:], EPS)
        onesb = const.tile([1, DH], bf16, name="onesb")
        nc.vector.memset(onesb[:], 1.0)
        b1pt = const.tile([P, NM1], f32, name="b1pt")
        nc.sync.dma_start(out=b1pt[:], in_=b1p_d)
        bias_tiles = {}
        for nm, dref in (("qb1", qb1_d), ("kb1", kb1_d), ("vb1", vb1_d),
                         ("q2b", q2b_d)):
            if dref is not None:
                t = const.tile([DH, H], f32, name=nm + "t")
                nc.sync.dma_start(out=t[:], in_=dref)
                bias_tiles[nm] = t
        for nm, dref in (("ob1", ob1_d), ("ob2", ob2_d), ("fb2", fb2_d)):
            if dref is not None:
                t = const.tile([P, D], f32, name=nm + "t")
                nc.sync.dma_start(out=t[:], in_=dref)
                bias_tiles[nm] = t

        stats = tc.alloc_tile_pool(name="stats", bufs=8)
        io = tc.alloc_tile_pool(name="io", bufs=5)
        h2p = tc.alloc_tile_pool(name="h2p", bufs=1)
        h2 = h2p.tile([P, NQT, D], f32, name="h2")

        def ln_block(xin, xT_dst, pt_pool, scratch_pool):
            """LayerNorm (scale/bias folded into weights) + bf16 transpose.

            xin [128, 640] fp32 sbuf; xT_dst(kt) -> [128, 128] bf16 AP."""
            st = stats.tile([P, 2, 6], f32, name="st", tag="st")
            nc.vector.bn_stats(st[:, 0, :], xin[:, 0:512])
            nc.vector.bn_stats(st[:, 1, :], xin[:, 512:D])
            mv = stats.tile([P, 2], f32, name="mv", tag="mv")
            nc.vector.bn_aggr(mv[:], st[:])
            rstd = stats.tile([P, 1], f32, name="rstd", tag="rstd")
            nc.scalar.activation(rstd[:], mv[:, 1:2], AF.Sqrt, bias=epst[:])
            nc.vector.reciprocal(rstd[:], rstd[:])
            mb = stats.tile([P, 1], f32, name="mb", tag="mb")
            nc.vector.tensor_scalar(
                out=mb[:], in0=mv[:, 0:1], scalar1=rstd[:], scalar2=-1.0,
                op0=Alu.mult, op1=Alu.mult)
            xn = scratch_pool.tile([P, D], bf16, name="xn", tag="xn")
            nc.gpsimd.tensor_scalar(
                out=xn[:], in0=xin, scalar1=rstd[:], scalar2=mb[:],
                op0=Alu.mult, op1=Alu.add)
            for kt in range(ND):
                ptile = pt_pool.tile([P, P], bf16, name="ptile", tag="pt")
                nc.tensor.transpose(ptile[:], xn[:, kt * P:(kt + 1) * P],
                                    identity[:])
                if kt % 2 == 0:
                    nc.scalar.copy(out=xT_dst(kt), in_=ptile[:])
                else:
                    nc.vector.tensor_copy(xT_dst(kt), ptile[:])

        # ================= attn1: LN1 + QKV projections =================
        k2Tp = tc.alloc_tile_pool(name="k2Tp", bufs=1)
        k2T = k2Tp.tile([P, H, ENCP], bf16, name="k2T")
        v2pp = tc.alloc_tile_pool(name="v2pp", bufs=1)
        v2p = v2pp.tile([ENC, H, VW], bf16, name="v2p")
        kTp = tc.alloc_tile_pool(name="kTp", bufs=1)
        kT = kTp.tile([P, H, KV], bf16, name="kT")
        vpp = tc.alloc_tile_pool(name="vpp", bufs=1)
        vp = vpp.tile([P, NKT, H, VW], bf16, name="vp")
        qTp = tc.alloc_tile_pool(name="qTp", bufs=1)
        qT = qTp.tile([P, H, S], bf16, name="qT")

        # own-frame raw hs stays resident (LN source + residual adds)
        hsop = tc.alloc_tile_pool(name="hsop", bufs=1)
        hso = hsop.tile([P, NQT, D], f32, name="hso")

        # prefetch attn1 weights as [P, ND, D] fp8
        wkvp = tc.alloc_tile_pool(name="wkvp", bufs=1)
        wk = wkvp.tile([P, ND, D], f8, name="wkt", tag="wkt")
        nc.sync.dma_start(out=wk[:],
                          in_=wk1_d.rearrange("(a p) d -> p a d", p=P))
        wv = wkvp.tile([P, ND, D], f8, name="wvt", tag="wvt")
        nc.sync.dma_start(out=wv[:],
                          in_=wv1_d.rearrange("(a p) d -> p a d", p=P))
        wq = wkvp.tile([P, ND, D], f8, name="wqt", tag="wqt")
        nc.sync.dma_start(out=wq[:],
                          in_=wq1_d.rearrange("(a p) d -> p a d", p=P))

        # ---- LN of kv frames (rows 1024:3072 of hs3) -> xkvT ----
        xkvp = tc.alloc_tile_pool(name="xkvp", bufs=1)
        xkvT = xkvp.tile([P, ND, KV], f8, name="xkvT")
        xowp = tc.alloc_tile_pool(name="xowp", bufs=1)
        xowT = xowp.tile([P, ND, S], f8, name="xowT")

        # encoder K/V built here: the LN phase is Vector-bound, PE has slack
        wk2p = tc.alloc_tile_pool(name="wk2p", bufs=1)
        wk2 = wk2p.tile([P, NE, D], f8, name="wk2t", tag="wk2t")
        nc.sync.dma_start(out=wk2[:],
                          in_=wk2_d.rearrange("(a p) d -> p a d", p=P))
        wv2 = wk2p.tile([P, NE, D], f8, name="wv2t", tag="wv2t")
        nc.sync.dma_start(out=wv2[:],
                          in_=wv2_d.rearrange("(a p) d -> p a d", p=P))
        encTp = tc.alloc_tile_pool(name="encTp", bufs=1)
        encT = encTp.tile([P, NE, ENCP], f8, name="encT")
        nc.vector.memset(encT[:], 0.0)
        encp = tc.alloc_tile_pool(name="encp", bufs=2)
        enc_sb = encp.tile([ENC, CROSS], f32, name="enc_sb")
        nc.sync.dma_start(out=enc_sb[:], in_=enc_d)
        enc_bf = encp.tile([ENC, CROSS], bf16, name="enc_bf")
        nc.vector.tensor_copy(enc_bf[:], enc_sb[:])

        ptp = tc.alloc_tile_pool(name="ptp", bufs=3, space=PSUM)
        pk2p = tc.alloc_tile_pool(name="pk2p", bufs=2, space=PSUM)
        pv2p = tc.alloc_tile_pool(name="pv2p", bufs=2, space=PSUM)
        for kt in range(NE):
            ptile = ptp.tile([P, P], bf16, name="ptile", tag="pt")
            nc.tensor.transpose(ptile[0:P, 0:ENC],
                                enc_bf[:, kt * P:(kt + 1) * P],
                                identity[0:ENC, 0:ENC])
            nc.vector.tensor_copy(encT[:, kt, 0:ENC], ptile[0:P, 0:ENC])
        for h in range(H):
            pk2 = pk2p.tile([P, ENCP], f32, name="pk2", tag="pk2")
            for j in range(NE // 2):
                nc.tensor.matmul(pk2[0:DH, 0:ENCP],
                                 wk2[:, 2 * j:2 * j + 2,
                                     h * DH:(h + 1) * DH],
                                 encT[:, 2 * j:2 * j + 2, :],
                                 start=(j == 0), stop=(j == NE // 2 - 1),
                                 perf_mode=DR)
            nc.scalar.copy(out=k2T[0:DH, h, :], in_=pk2[0:DH, :])
        pv20 = pv2p.tile([P, 320], f32, name="pv20", tag="pv2")
        pv21 = pv2p.tile([P, 320], f32, name="pv21", tag="pv2")
        for j in range(NE // 2):
            nc.tensor.matmul(pv20[0:ENC, :],
                             encT[:, 2 * j:2 * j + 2, 0:ENC],
                             wv2[:, 2 * j:2 * j + 2, 0:320],
                             start=(j == 0), stop=(j == NE // 2 - 1),
                             perf_mode=DR)
            nc.tensor.matmul(pv21[0:ENC, :],
                             encT[:, 2 * j:2 * j + 2, 0:ENC],
                             wv2[:, 2 * j:2 * j + 2, 320:640],
                             start=(j == 0), stop=(j == NE // 2 - 1),
                             perf_mode=DR)
        nc.vector.tensor_copy(
            v2p[:, 0:4, 0:DH],
            pv20[0:ENC, :].rearrange("p (a b) -> p a b", b=DH))
        nc.vector.tensor_copy(
            v2p[:, 4:8, 0:DH],
            pv21[0:ENC, :].rearrange("p (a b) -> p a b", b=DH))
        nc.vector.memset(v2p[:, :, DH:DH + 1], 1.0)

        for t in range(NKT):
            xt = io.tile([P, D], f32, name="xt", tag="io")
            nc.sync.dma_start(out=xt[:],
                              in_=hs3_d[(NQT + t) * P:(NQT + t + 1) * P, :])
            ln_block(xt[:], lambda kt, t=t: xkvT[:, kt, t * P:(t + 1) * P],
                     ptp, io)
        # ---- LN of own frame -> xowT ----
        nc.sync.dma_start(out=hso[:],
                          in_=hs3_d[0:S, :].rearrange("(t p) d -> p t d", p=P))
        for t in range(NQT):
            ln_block(hso[:, t, :],
                     lambda kt, t=t: xowT[:, kt, t * P:(t + 1) * P], ptp, io)
        pv2p.release()
        pk2p.release()
        ptp.release()
        encp.release()
        encTp.release()
        wk2p.release()

        # ---- projections ----
        pjp = tc.alloc_tile_pool(name="pjp", bufs=3, space=PSUM)
        pvp = tc.alloc_tile_pool(name="pvp", bufs=2, space=PSUM)

        # K: kT[dh, h, kv]
        for h in range(H):
            for c2 in range(2):
                pk = pjp.tile([P, S], f32, name="pk", tag="pj")
                for j in range(2):
                    for half in range(2):
                        nc.tensor.matmul(
                            pk[0:DH, half * 512:(half + 1) * 512],
                            wk[:, 2 * j:2 * j + 2, h * DH:(h + 1) * DH],
                            xkvT[:, 2 * j:2 * j + 2,
                                 c2 * S + half * 512:c2 * S + (half + 1) * 512],
                            start=(j == 0), stop=False, perf_mode=DR)
                for half in range(2):
                    nc.tensor.matmul(
                        pk[0:DH, half * 512:(half + 1) * 512],
                        wk[:, ND - 1, h * DH:(h + 1) * DH],
                        xkvT[:, ND - 1, c2 * S + half * 512:
                             c2 * S + (half + 1) * 512],
                        start=False, stop=(half == 1))
                if has_kb1:
                    nc.vector.tensor_scalar_add(
                        pk[0:DH, :], pk[0:DH, :],
                        bias_tiles["kb1"][:, h:h + 1])
                if (h + c2) % 2 == 0:
                    nc.scalar.copy(out=kT[0:DH, h, c2 * S:(c2 + 1) * S],
                                   in_=pk[0:DH, :])
                else:
                    nc.vector.tensor_copy(kT[0:DH, h, c2 * S:(c2 + 1) * S],
                                          pk[0:DH, :])

        # V: vp[kv, kvt, h, 0:80] + ones col at 80
        for m in range(NKT):
            pv0 = pvp.tile([P, 320], f32, name="pv0", tag="pv")
            pv1 = pvp.tile([P, 320], f32, name="pv1", tag="pv")
            for j in range(2):
                nc.tensor.matmul(pv0[:],
                                 xkvT[:, 2 * j:2 * j + 2, m * P:(m + 1) * P],
                                 wv[:, 2 * j:2 * j + 2, 0:320],
                                 start=(j == 0), stop=False, perf_mode=DR)
                nc.tensor.matmul(pv1[:],
                                 xkvT[:, 2 * j:2 * j + 2, m * P:(m + 1) * P],
                                 wv[:, 2 * j:2 * j + 2, 320:640],
                                 start=(j == 0), stop=False, perf_mode=DR)
            nc.tensor.matmul(pv0[:], xkvT[:, ND - 1, m * P:(m + 1) * P],
                             wv[:, ND - 1, 0:320], start=False, stop=True)
            nc.tensor.matmul(pv1[:], xkvT[:, ND - 1, m * P:(m + 1) * P],
                             wv[:, ND - 1, 320:640], start=False, stop=True)
            vsl = vp[:, m, :, :]
            nc.vector.tensor_copy(
                vsl[:, 0:4, 0:DH], pv0[:].rearrange("p (a b) -> p a b", b=DH))
            nc.vector.tensor_copy(
                vsl[:, 4:8, 0:DH], pv1[:].rearrange("p (a b) -> p a b", b=DH))
            nc.vector.memset(vsl[:, :, DH:DH + 1], 1.0)

        # Q: qT[dh, h, tok]
        for h in range(H):
            pq = pjp.tile([P, S], f32, name="pq", tag="pj")
            for j in range(2):
                for c in range(2):
                    nc.tensor.matmul(
                        pq[0:DH, c * 512:(c + 1) * 512],
                        wq[:, 2 * j:2 * j + 2, h * DH:(h + 1) * DH],
                        xowT[:, 2 * j:2 * j + 2, c * 512:(c + 1) * 512],
                        start=(j == 0), stop=False, perf_mode=DR)
            for c in range(2):
                nc.tensor.matmul(
                    pq[0:DH, c * 512:(c + 1) * 512],
                    wq[:, ND - 1, h * DH:(h + 1) * DH],
                    xowT[:, ND - 1, c * 512:(c + 1) * 512],
                    start=False, stop=(c == 1))
            if has_qb1:
                nc.vector.tensor_scalar_add(
                    pq[0:DH, :], pq[0:DH, :], bias_tiles["qb1"][:, h:h + 1])
            if h % 2 == 0:
                nc.scalar.copy(out=qT[0:DH, h, :], in_=pq[0:DH, :])
            else:
                nc.vector.tensor_copy(qT[0:DH, h, :], pq[0:DH, :])
        pvp.release()
        pjp.release()
        xowp.release()
        xkvp.release()
        wkvp.release()

        # ================= attn1: attention =================
        o1pp = tc.alloc_tile_pool(name="o1pp", bufs=1)
        o1pt = o1pp.tile([P, H, D], bf16, name="o1pt")
        nc.sync.dma_start(out=o1pt[0:DH, :, :],
                          in_=o1p_d.rearrange("h p d -> p h d"))
        sbavp = tc.alloc_tile_pool(name="sbavp", bufs=1)
        sbavs = []
        for h in range(H):
            sbavs.append(sbavp.tile([DH + 1, S], bf16, name=f"sbav{h}",
                                    tag=f"sbav{h}"))
        denp = tc.alloc_tile_pool(name="denp", bufs=4)
        expp = tc.alloc_tile_pool(name="expp", bufs=3)
        psp = tc.alloc_tile_pool(name="psp", bufs=2, space=PSUM)
        pavp = tc.alloc_tile_pool(name="pavp", bufs=1, space=PSUM)
        pbp = tc.alloc_tile_pool(name="pbp", bufs=1, space=PSUM)

        for h in range(H):
            pav = pavp.tile([P, S], f32, name="pav", tag="pav")
            for kvt in range(NKT):
                ps = psp.tile([P, S], f32, name="ps", tag="ps")
                for half in range(2):
                    nc.tensor.matmul(
                        ps[:, half * 512:(half + 1) * 512],
                        kT[0:DH, h, kvt * P:(kvt + 1) * P],
                        qT[0:DH, h, half * 512:(half + 1) * 512],
                        start=True, stop=True)
                ex = expp.tile([P, S], bf16, name="ex", tag="exp")
                nc.scalar.activation(ex[:], ps[:], AF.Exp, scale=ESCALE)
                for half in range(2):
                    nc.tensor.matmul(
                        pav[0:DH + 1, half * 512:(half + 1) * 512],
                        vp[:, kvt, h, 0:DH + 1],
                        ex[:, half * 512:(half + 1) * 512],
                        start=(kvt == 0), stop=(kvt == NKT - 1))
            nc.vector.tensor_copy(sbavs[h][:], pav[0:DH + 1, :])
            den = denp.tile([1, S], bf16, name="den", tag="den")
            nc.sync.dma_start(out=den[:], in_=sbavs[h][DH:DH + 1, :])
            pb = pbp.tile([P, S], f32, name="pb", tag="pb")
            for c in range(2):
                nc.tensor.matmul(pb[0:DH, c * 512:(c + 1) * 512], onesb[:],
                                 den[:, c * 512:(c + 1) * 512],
                                 start=True, stop=True)
            nc.vector.reciprocal_approx_fast(pb[0:DH, :], pb[0:DH, :])
            nc.vector.tensor_mul(sbavs[h][0:DH, :], sbavs[h][0:DH, :],
                                 pb[0:DH, :])
            if has_vb1:
                nc.vector.tensor_scalar_add(
                    sbavs[h][0:DH, :], sbavs[h][0:DH, :],
                    bias_tiles["vb1"][:, h:h + 1])
        pbp.release()
        pavp.release()
        psp.release()
        expp.release()
        denp.release()

        # o1 projection + residual
        pop = tc.alloc_tile_pool(name="pop", bufs=4, space=PSUM)
        for t in range(NQT):
            po0 = pop.tile([P, 320], f32, name="po0", tag="po")
            po1 = pop.tile([P, 320], f32, name="po1", tag="po")
            for h in range(H):
                nc.tensor.matmul(po0[:], sbavs[h][0:DH, t * P:(t + 1) * P],
                                 o1pt[0:DH, h, 0:320],
                                 start=(h == 0), stop=(h == H - 1))
                nc.tensor.matmul(po1[:], sbavs[h][0:DH, t * P:(t + 1) * P],
                                 o1pt[0:DH, h, 320:640],
                                 start=(h == 0), stop=(h == H - 1))
            nc.vector.tensor_add(h2[:, t, 0:320], po0[:], hso[:, t, 0:320])
            nc.vector.tensor_add(h2[:, t, 320:640], po1[:],
                                 hso[:, t, 320:640])
            if has_ob1:
                nc.vector.tensor_add(h2[:, t, :], h2[:, t, :],
                                     bias_tiles["ob1"][:])
        pop.release()
        sbavp.release()
        o1pp.release()
        hsop.release()
        qTp.release()
        vpp.release()
        kTp.release()
        h3p = tc.alloc_tile_pool(name="h3p", bufs=1)
        h3 = h3p.tile([P, NQT, D], f32, name="h3")

        # ================= attn2: cross attention =================
        q2Tp = tc.alloc_tile_pool(name="q2Tp", bufs=1)
        q2T = q2Tp.tile([P, H, S], bf16, name="q2T")
        x2p = tc.alloc_tile_pool(name="x2p", bufs=1)
        x2T = x2p.tile([P, ND, S], f8, name="x2T")
        sbav2p = tc.alloc_tile_pool(name="sbav2p", bufs=1)
        den2rp = tc.alloc_tile_pool(name="den2rp", bufs=1)
        dens2 = []
        sbavs2 = []
        for h in range(H):
            dens2.append(den2rp.tile([1, S], bf16, name=f"d2en{h}",
                                     tag=f"d2en{h}"))
            sbavs2.append(sbav2p.tile([DH + 1, S], bf16, name=f"sbav2{h}",
                                      tag=f"sbav2{h}"))
        pjp2 = tc.alloc_tile_pool(name="pjp2", bufs=2, space=PSUM)

        wq2p = tc.alloc_tile_pool(name="wq2p", bufs=1)
        wq2 = wq2p.tile([P, ND, D], f8, name="wq2t", tag="wq2t")
        nc.sync.dma_start(out=wq2[:],
                          in_=wq2_d.rearrange("(a p) d -> p a d", p=P))

        ptp2 = tc.alloc_tile_pool(name="ptp2", bufs=3, space=PSUM)
        for t in range(NQT):
            ln_block(h2[:, t, :],
                     lambda kt, t=t: x2T[:, kt, t * P:(t + 1) * P], ptp2, io)
        ptp2.release()

        # q2 projection
        for h in range(H):
            pq = pjp2.tile([P, S], f32, name="pq2", tag="pj2")
            for j in range(2):
                for c in range(2):
                    nc.tensor.matmul(
                        pq[0:DH, c * 512:(c + 1) * 512],
                        wq2[:, 2 * j:2 * j + 2, h * DH:(h + 1) * DH],
                        x2T[:, 2 * j:2 * j + 2, c * 512:(c + 1) * 512],
                        start=(j == 0), stop=False, perf_mode=DR)
            for c in range(2):
                nc.tensor.matmul(
                    pq[0:DH, c * 512:(c + 1) * 512],
                    wq2[:, ND - 1, h * DH:(h + 1) * DH],
                    x2T[:, ND - 1, c * 512:(c + 1) * 512],
                    start=False, stop=(c == 1))
            if has_q2b:
                nc.vector.tensor_scalar_add(
                    pq[0:DH, :], pq[0:DH, :], bias_tiles["q2b"][:, h:h + 1])
            if h % 2 == 0:
                nc.scalar.copy(out=q2T[0:DH, h, :], in_=pq[0:DH, :])
            else:
                nc.vector.tensor_copy(q2T[0:DH, h, :], pq[0:DH, :])

        wq2p.release()

        # attention 2
        exp2p = tc.alloc_tile_pool(name="exp2p", bufs=3)
        pav2p = tc.alloc_tile_pool(name="pav2p", bufs=2, space=PSUM)
        for h in range(H):
            ps2 = pjp2.tile([P, S], f32, name="ps2", tag="pj2")
            for c in range(2):
                nc.tensor.matmul(ps2[0:ENC, c * 512:(c + 1) * 512],
                                 k2T[0:DH, h, 0:ENC],
                                 q2T[0:DH, h, c * 512:(c + 1) * 512],
                                 start=True, stop=True)
            ex2 = exp2p.tile([P, S], bf16, name="ex2", tag="exp2")
            nc.scalar.activation(ex2[0:ENC, :], ps2[0:ENC, :], AF.Exp,
                                 scale=ESCALE)
            pav2 = pav2p.tile([P, S], f32, name="pav2", tag="pav2")
            for c in range(2):
                nc.tensor.matmul(pav2[0:DH + 1, c * 512:(c + 1) * 512],
                                 v2p[:, h, 0:DH + 1],
                                 ex2[0:ENC, c * 512:(c + 1) * 512],
                                 start=True, stop=True)
            nc.vector.tensor_copy(sbavs2[h][:], pav2[0:DH + 1, :])
            nc.sync.dma_start(out=dens2[h][:], in_=sbavs2[h][DH:DH + 1, :])
        pav2p.release()
        exp2p.release()

        # normalize + o2 projection + residual -> h3
        o2pp = tc.alloc_tile_pool(name="o2pp", bufs=1)
        o2pt = o2pp.tile([P, H, D], bf16, name="o2pt")
        nc.sync.dma_start(out=o2pt[0:DH, :, :],
                          in_=o2p_d.rearrange("h p d -> p h d"))
        pop2 = tc.alloc_tile_pool(name="pop2", bufs=4, space=PSUM)
        for h in range(H):
            pb = pjp2.tile([P, S], f32, name="pb2", tag="pj2")
            for c in range(2):
                nc.tensor.matmul(pb[0:DH, c * 512:(c + 1) * 512], onesb[:],
                                 dens2[h][:, c * 512:(c + 1) * 512],
                                 start=True, stop=True)
            nc.vector.reciprocal_approx_fast(pb[0:DH, :], pb[0:DH, :])
            nc.vector.tensor_mul(sbavs2[h][0:DH, :], sbavs2[h][0:DH, :],
                                 pb[0:DH, :])
        for t in range(NQT):
            po0 = pop2.tile([P, 320], f32, name="po20", tag="po2")
            po1 = pop2.tile([P, 320], f32, name="po21", tag="po2")
            for h in range(H):
                nc.tensor.matmul(po0[:], sbavs2[h][0:DH, t * P:(t + 1) * P],
                                 o2pt[0:DH, h, 0:320],
                                 start=(h == 0), stop=(h == H - 1))
                nc.tensor.matmul(po1[:], sbavs2[h][0:DH, t * P:(t + 1) * P],
                                 o2pt[0:DH, h, 320:640],
                                 start=(h == 0), stop=(h == H - 1))
            nc.vector.tensor_add(h3[:, t, 0:320], po0[:], h2[:, t, 0:320])
            nc.vector.tensor_add(h3[:, t, 320:640], po1[:], h2[:, t, 320:640])
            if has_ob2:
                nc.vector.tensor_add(h3[:, t, :], h3[:, t, :],
                                     bias_tiles["ob2"][:])
        pop2.release()
        o2pp.release()
        pjp2.release()
        den2rp.release()
        sbav2p.release()
        x2p.release()
        q2Tp.release()

        # ================= FFN (geglu) =================
        hgTp = tc.alloc_tile_pool(name="hgTp", bufs=1)
        hgT = hgTp.tile([P, NK2, S], f8, name="hgT")
        x3p = tc.alloc_tile_pool(name="x3p", bufs=1)
        x3T = x3p.tile([P, ND, S], f8, name="x3T")
        ptp3 = tc.alloc_tile_pool(name="ptp3", bufs=3, space=PSUM)
        for t in range(NQT):
            ln_block(h3[:, t, :],
                     lambda kt, t=t: x3T[:, kt, t * P:(t + 1) * P], ptp3, io)
        ptp3.release()

        w1pp = tc.alloc_tile_pool(name="w1pp", bufs=6)
        ggp = tc.alloc_tile_pool(name="ggp", bufs=3)
        pw1 = tc.alloc_tile_pool(name="pw1", bufs=4, space=PSUM)
        for mp in range(NK2):
            wh = w1pp.tile([P, ND, P], f8, name="wh", tag="w1")
            nc.sync.dma_start(out=wh[:], in_=w1p_d[mp])
            wg = w1pp.tile([P, ND, P], f8, name="wg", tag="w1")
            nc.sync.dma_start(out=wg[:], in_=w1p_d[mp + NK2])
            ph = pw1.tile([P, S], f32, name="ph", tag="pw1")
            pg = pw1.tile([P, S], f32, name="pg", tag="pw1")
            for j in range(2):
                for c in range(2):
                    nc.tensor.matmul(ph[:, c * 512:(c + 1) * 512],
                                     wh[:, 2 * j:2 * j + 2, :],
                                     x3T[:, 2 * j:2 * j + 2,
                                         c * 512:(c + 1) * 512],
                                     start=(j == 0), stop=False, perf_mode=DR)
            for c in range(2):
                nc.tensor.matmul(ph[:, c * 512:(c + 1) * 512],
                                 wh[:, ND - 1, :],
                                 x3T[:, ND - 1, c * 512:(c + 1) * 512],
                                 start=False, stop=(c == 1))
            for j in range(2):
                for c in range(2):
                    nc.tensor.matmul(pg[:, c * 512:(c + 1) * 512],
                                     wg[:, 2 * j:2 * j + 2, :],
                                     x3T[:, 2 * j:2 * j + 2,
                                         c * 512:(c + 1) * 512],
                                     start=(j == 0), stop=False, perf_mode=DR)
            for c in range(2):
                nc.tensor.matmul(pg[:, c * 512:(c + 1) * 512],
                                 wg[:, ND - 1, :],
                                 x3T[:, ND - 1, c * 512:(c + 1) * 512],
                                 start=False, stop=(c == 1))
            gg = ggp.tile([P, S], bf16, name="gg", tag="gg")
            nc.scalar.activation(gg[:], pg[:], AF.Gelu_apprx_tanh,
                                 bias=b1pt[:, mp + NK2:mp + NK2 + 1],
                                 scale=1.0 / SW1)
            nc.vector.scalar_tensor_tensor(
                out=hgT[:, mp, :],
                in0=ph[:], scalar=b1pt[:, mp:mp + 1], in1=gg[:],
                op0=Alu.add, op1=Alu.mult)
        pw1.release()
        ggp.release()
        w1pp.release()
        x3p.release()

        w2pp = tc.alloc_tile_pool(name="w2pp", bufs=4)
        pw2 = tc.alloc_tile_pool(name="pw2", bufs=8, space=PSUM)
        for th in range(2):
            pf = []
            for tt in range(4):
                pf.append((pw2.tile([P, 320], f32, name=f"pf{tt}a", tag="pw2"),
                           pw2.tile([P, 320], f32, name=f"pf{tt}b", tag="pw2")))
            for kj in range(NK2 // 2):
                w2t = w2pp.tile([P, 2, D], f8, name="w2t", tag="w2")
                nc.sync.dma_start(
                    out=w2t[:],
                    in_=w2p_d[2 * kj:2 * kj + 2]
                    .rearrange("a p d -> p a d"))
                for tt in range(4):
                    t = th * 4 + tt
                    nc.tensor.matmul(pf[tt][0][:],
                                     hgT[:, 2 * kj:2 * kj + 2,
                                         t * P:(t + 1) * P],
                                     w2t[:, :, 0:320],
                                     start=(kj == 0), stop=(kj == NK2 // 2 - 1),
                                     perf_mode=DR)
                    nc.tensor.matmul(pf[tt][1][:],
                                     hgT[:, 2 * kj:2 * kj + 2,
                                         t * P:(t + 1) * P],
                                     w2t[:, :, 320:640],
                                     start=(kj == 0), stop=(kj == NK2 // 2 - 1),
                                     perf_mode=DR)
            for tt in range(4):
                t = th * 4 + tt
                ot = io.tile([P, D], f32, name="ot", tag="io")
                nc.vector.scalar_tensor_tensor(
                    out=ot[:, 0:320], in0=pf[tt][0][:], scalar=ODESC,
                    in1=h3[:, t, 0:320], op0=Alu.mult, op1=Alu.add)
                nc.vector.scalar_tensor_tensor(
                    out=ot[:, 320:640], in0=pf[tt][1][:], scalar=ODESC,
                    in1=h3[:, t, 320:640], op0=Alu.mult, op1=Alu.add)
                if has_fb2:
                    nc.vector.tensor_add(ot[:], ot[:], bias_tiles["fb2"][:])
                nc.gpsimd.dma_start(out=out_d[t * P:(t + 1) * P, :], in_=ot[:])
        pw2.release()
        w2pp.release()
        hgTp.release()

        h3p.release()
        v2pp.release()
        k2Tp.release()
        h2p.release()
        io.release()
        stats.release()
        const.release()

    nc.compile()
    return nc


def _prep_inputs(inputs):
    import ml_dtypes

    f32 = np.float32
    bf16 = ml_dtypes.bfloat16
    f8 = ml_dtypes.float8_e4m3
    g = {k: np.asarray(v) for k, v in inputs.items()}
    hs = np.ascontiguousarray(g["hidden_states"], f32)
    enc = np.ascontiguousarray(g["encoder_hidden_states"], f32)
    f = int(g["video_length"])
    assert hs.shape == (F, S, D) and enc.shape == (F, ENC, CROSS) and f == F

    ln1w, ln1b = g["ln1_w"].astype(f32), g["ln1_b"].astype(f32)
    ln2w, ln2b = g["ln2_w"].astype(f32), g["ln2_b"].astype(f32)
    ln3w, ln3b = g["ln3_w"].astype(f32), g["ln3_b"].astype(f32)
    q1, k1, v1 = (g[n].astype(f32) for n in ("q1", "k1", "v1"))
    o1w, o1b = g["o1_w"].astype(f32), g["o1_b"].astype(f32)
    q2, k2, v2 = (g[n].astype(f32) for n in ("q2", "k2", "v2"))
    o2w, o2b = g["o2_w"].astype(f32), g["o2_b"].astype(f32)
    w1, b1 = g["ff_w1"].astype(f32), g["ff_b1"].astype(f32)
    w2, b2 = g["ff_w2"].astype(f32), g["ff_b2"].astype(f32)

    b1f = (b1 + ln3b @ w1).reshape(NM1, P).T.copy()
    b1f[:, 0:NK2] *= SW1
    shared = {
        "wq1": np.ascontiguousarray(q1 * ln1w[:, None] * SW).astype(f8),
        "wk1": np.ascontiguousarray(k1 * ln1w[:, None] * SW).astype(f8),
        "wv1": np.ascontiguousarray(v1 * ln1w[:, None] * SW).astype(f8),
        "o1p": np.ascontiguousarray(o1w.reshape(H, DH, D) / SW).astype(bf16),
        "wq2": np.ascontiguousarray(q2 * ln2w[:, None] * SW).astype(f8),
        "wk2": np.ascontiguousarray(k2 * SW).astype(f8),
        "wv2": np.ascontiguousarray(v2 * SW).astype(f8),
        "o2p": np.ascontiguousarray(o2w.reshape(H, DH, D) / SW).astype(bf16),
        "w1p": np.ascontiguousarray(
            (w1 * ln3w[:, None] * SW1).reshape(ND, P, NM1, P)
            .transpose(2, 1, 0, 3)).astype(f8),
        "b1p": np.ascontiguousarray(b1f),
        "w2p": np.ascontiguousarray(w2.reshape(NK2, P, D) * SW2).astype(f8),
    }

    qb1 = (ln1b @ q1) * SW
    kb1 = (ln1b @ k1) * SW
    vb1 = (ln1b @ v1) * SW
    q2b = (ln2b @ q2) * SW
    flags = (
        bool(np.any(qb1)), bool(np.any(kb1)), bool(np.any(vb1)),
        bool(np.any(o1b)), bool(np.any(q2b)), bool(np.any(o2b)),
        bool(np.any(b2)),
    )
    has_qb1, has_kb1, has_vb1, has_ob1, has_q2b, has_ob2, has_fb2 = flags
    if has_qb1:
        shared["qb1"] = np.ascontiguousarray(qb1.reshape(H, DH).T)
    if has_kb1:
        shared["kb1"] = np.ascontiguousarray(kb1.reshape(H, DH).T)
    if has_vb1:
        shared["vb1"] = np.ascontiguousarray(vb1.reshape(H, DH).T)
    if has_q2b:
        shared["q2b"] = np.ascontiguousarray(q2b.reshape(H, DH).T)
    if has_ob1:
        shared["ob1"] = np.ascontiguousarray(np.broadcast_to(o1b, (P, D)))
    if has_ob2:
        shared["ob2"] = np.ascontiguousarray(np.broadcast_to(o2b, (P, D)))
    if has_fb2:
        shared["fb2"] = np.ascontiguousarray(np.broadcast_to(b2, (P, D)))

    former = [0] + list(range(F - 1))
    in_maps = []
    for i in range(F):
        m = dict(shared)
        m["hs3"] = np.ascontiguousarray(
            np.concatenate([hs[i], hs[0], hs[former[i]]], axis=0))
        m["enc"] = np.ascontiguousarray(enc[i])
        in_maps.append(m)
    return flags, in_maps


def get_program(flags):
    if flags not in _PROGRAM_CACHE:
        _PROGRAM_CACHE[flags] = _build_program(flags)
    return _PROGRAM_CACHE[flags]


def run(inputs, trace=False):
    from concourse.bass_utils import run_bass_kernel_spmd

    flags, in_maps = _prep_inputs(inputs)
    nc = get_program(flags)
    res = run_bass_kernel_spmd(nc, in_maps, core_ids=list(range(F)),
                               trace=trace)
    out = np.stack([r["out"] for r in res.results], axis=0)
    return out.astype(np.float32), res


def kernel(**inputs):
    out, _ = run(inputs, trace=False)
    return out


# revision 35
# speedup vs baseline: 1.0063x; 1.0063x over previous
"""Trainium2 Bass kernel for a sparse-causal-attention BasicTransformerBlock.

Sharding: pure data-parallel over the 8 video frames (batch=1, f=8) - one
frame per NeuronCore, zero collectives.  Each core receives its own frame
plus frame 0 and the previous frame (the sparse-causal KV sources) and
recomputes LN1 + K/V projections for those locally.

v2 design notes (vs the first working version):
  - All PE operands bf16 (weights pre-cast on host): enables fast-weight-load
    and halves weight DMA.  PSUM accumulation stays fp32.
  - Loops ordered so consecutive matmuls share the stationary operand
    (one LDWEIGHTS per weight tile, streaming 2x512 columns).
  - Attention runs kvt-major over the full 1024-token q range: per (h, kvt)
    one [128kv, 1024q] score tile -> one exp -> AV accumulation.  Softmax
    denominator comes from a ones-column appended to V.
  - Softmax normalization: denominator rows for all 8 heads are collected
    into one [8, 1024] tile, a single reciprocal_approx_fast inverts them,
    and a K=1 PE matmul broadcasts each row across partitions (no DRAM
    round-trips, no single-lane iterative divides).
  - LayerNorm normalize runs on the Vector engine (tensor_scalar) with a
    bf16 output feeding PE transposes; LN scale vectors are folded into the
    consuming projection weights on the host.
"""

import numpy as np

P = 128
S = 1024          # tokens per frame
D = 640
H = 8
DH = 80
KV = 2 * S        # sparse-causal kv tokens (first frame + prev frame)
ENC = 77
ENCP = 80         # padded to 16-byte stride for fp8 DoubleRow
CROSS = 768
FFI = 2560
NQT = S // P      # 8 token tiles
NKT = KV // P     # 16 kv token tiles
ND = D // P       # 5 dim tiles
NE = CROSS // P   # 6 encoder-dim tiles
NM1 = 2 * FFI // P  # 40 ff_w1 out tiles
NK2 = FFI // P      # 20 ff_w2 k tiles
VW = 88           # per-head V stride in vp tiles (80 dh + 1 ones + 7 pad)
SCALE = DH ** -0.5
SW = 16.0         # fp8 weight scale for qkv / cross projections
SW1 = 16.0        # fp8 scale for ff_w1
SW2 = 32.0        # fp8 scale for ff_w2
ESCALE = SCALE / (SW * SW)   # exp scale (descales q and k fp8 weight scales)
ODESC = 1.0 / (SW1 * SW2)    # ffn output descale
EPS = 1e-5
F = 8

_PROGRAM_CACHE = {}


def _build_program(flags):
    import concourse.bass as bass
    import concourse.tile as tile
    from concourse import bacc, mybir
    from concourse.masks import make_identity

    f32 = mybir.dt.float32
    f32r = mybir.dt.float32r
    bf16 = mybir.dt.bfloat16
    f8 = mybir.dt.float8e4
    DR = mybir.MatmulPerfMode.DoubleRow
    AF = mybir.ActivationFunctionType
    Alu = mybir.AluOpType
    PSUM = bass.MemorySpace.PSUM

    (has_qb1, has_kb1, has_vb1, has_ob1, has_q2b, has_ob2, has_fb2) = flags

    nc = bacc.Bacc(None, target_bir_lowering=False)

    hs3_d = nc.dram_tensor("hs3", [3 * S, D], f32, kind="ExternalInput")[:]
    enc_d = nc.dram_tensor("enc", [ENC, CROSS], f32, kind="ExternalInput")[:]
    wq1_d = nc.dram_tensor("wq1", [D, D], f8, kind="ExternalInput")[:]
    wk1_d = nc.dram_tensor("wk1", [D, D], f8, kind="ExternalInput")[:]
    wv1_d = nc.dram_tensor("wv1", [D, D], f8, kind="ExternalInput")[:]
    o1p_d = nc.dram_tensor("o1p", [H, DH, D], bf16, kind="ExternalInput")[:]
    wq2_d = nc.dram_tensor("wq2", [D, D], f8, kind="ExternalInput")[:]
    wk2_d = nc.dram_tensor("wk2", [CROSS, D], f8, kind="ExternalInput")[:]
    wv2_d = nc.dram_tensor("wv2", [CROSS, D], f8, kind="ExternalInput")[:]
    o2p_d = nc.dram_tensor("o2p", [H, DH, D], bf16, kind="ExternalInput")[:]
    w1p_d = nc.dram_tensor("w1p", [NM1, P, ND, P], f8, kind="ExternalInput")[:]
    b1p_d = nc.dram_tensor("b1p", [P, NM1], f32, kind="ExternalInput")[:]
    w2p_d = nc.dram_tensor("w2p", [NK2, P, D], f8, kind="ExternalInput")[:]
    out_d = nc.dram_tensor("out", [S, D], f32, kind="ExternalOutput")[:]

    qb1_d = kb1_d = vb1_d = q2b_d = None
    ob1_d = ob2_d = fb2_d = None
    if has_qb1:
        qb1_d = nc.dram_tensor("qb1", [DH, H], f32, kind="ExternalInput")[:]
    if has_kb1:
        kb1_d = nc.dram_tensor("kb1", [DH, H], f32, kind="ExternalInput")[:]
    if has_vb1:
        vb1_d = nc.dram_tensor("vb1", [DH, H], f32, kind="ExternalInput")[:]
    if has_q2b:
        q2b_d = nc.dram_tensor("q2b", [DH, H], f32, kind="ExternalInput")[:]
    if has_ob1:
        ob1_d = nc.dram_tensor("ob1", [P, D], f32, kind="ExternalInput")[:]
    if has_ob2:
        ob2_d = nc.dram_tensor("ob2", [P, D], f32, kind="ExternalInput")[:]
    if has_fb2:
        fb2_d = nc.dram_tensor("fb2", [P, D], f32, kind="ExternalInput")[:]

    with tile.TileContext(nc) as tc:
        # ---------- whole-kernel pools ----------
        const = tc.alloc_tile_pool(name="const", bufs=1)
        identity = const.tile([P, P], bf16, name="identity")
        make_identity(nc, identity)
        epst = const.tile([P, 1], f32, name="epst")
        nc.vector.memset(epst[:], EPS)
        onesb = const.tile([1, DH], bf16, name="onesb")
        nc.vector.memset(onesb[:], 1.0)
        b1pt = const.tile([P, NM1], f32, name="b1pt")
        nc.sync.dma_start(out=b1pt[:], in_=b1p_d)
        bias_tiles = {}
        for nm, dref in (("qb1", qb1_d), ("kb1", kb1_d), ("vb1", vb1_d),
                         ("q2b", q2b_d)):
            if dref is not None:
                t = const.tile([DH, H], f32, name=nm + "t")
                nc.sync.dma_start(out=t[:], in_=dref)
                bias_tiles[nm] = t
        for nm, dref in (("ob1", ob1_d), ("ob2", ob2_d), ("fb2", fb2_d)):
            if dref is not None:
                t = const.tile([P, D], f32, name=nm + "t")
                nc.sync.dma_start(out=t[:], in_=dref)
                bias_tiles[nm] = t

        stats = tc.alloc_tile_pool(name="stats", bufs=8)
        io = tc.alloc_tile_pool(name="io", bufs=5)
        h2p = tc.alloc_tile_pool(name="h2p", bufs=1)
        h2 = h2p.tile([P, NQT, D], f32, name="h2")

        def ln_block(xin, xT_dst, pt_pool, scratch_pool):
            """LayerNorm (scale/bias folded into weights) + bf16 transpose.

            xin [128, 640] fp32 sbuf; xT_dst(kt) -> [128, 128] bf16 AP."""
            st = stats.tile([P, 2, 6], f32, name="st", tag="st")
            nc.vector.bn_stats(st[:, 0, :], xin[:, 0:512])
            nc.vector.bn_stats(st[:, 1, :], xin[:, 512:D])
            mv = stats.tile([P, 2], f32, name="mv", tag="mv")
            nc.vector.bn_aggr(mv[:], st[:])
            rstd = stats.tile([P, 1], f32, name="rstd", tag="rstd")
            nc.scalar.activation(rstd[:], mv[:, 1:2], AF.Sqrt, bias=epst[:])
            nc.vector.reciprocal(rstd[:], rstd[:])
            mb = stats.tile([P, 1], f32, name="mb", tag="mb")
            nc.vector.tensor_scalar(
                out=mb[:], in0=mv[:, 0:1], scalar1=rstd[:], scalar2=-1.0,
                op0=Alu.mult, op1=Alu.mult)
            xn = scratch_pool.tile([P, D], bf16, name="xn", tag="xn")
            nc.gpsimd.tensor_scalar(
                out=xn[:], in0=xin, scalar1=rstd[:], scalar2=mb[:],
                op0=Alu.mult, op1=Alu.add)
            for kt in range(ND):
                ptile = pt_pool.tile([P, P], bf16, name="ptile", tag="pt")
                nc.tensor.transpose(ptile[:], xn[:, kt * P:(kt + 1) * P],
                                    identity[:])
                if kt % 2 == 0:
                    nc.scalar.copy(out=xT_dst(kt), in_=ptile[:])
                else:
                    nc.vector.tensor_copy(xT_dst(kt), ptile[:])

        # ================= attn1: LN1 + QKV projections =================
        k2Tp = tc.alloc_tile_pool(name="k2Tp", bufs=1)
        k2T = k2Tp.tile([P, H, ENCP], bf16, name="k2T")
        v2pp = tc.alloc_tile_pool(name="v2pp", bufs=1)
        v2p = v2pp.tile([ENC, H, VW], bf16, name="v2p")
        kTp = tc.alloc_tile_pool(name="kTp", bufs=1)
        kT = kTp.tile([P, H, KV], bf16, name="kT")
        vpp = tc.alloc_tile_pool(name="vpp", bufs=1)
        vp = vpp.tile([P, NKT, H, VW], bf16, name="vp")
        qTp = tc.alloc_tile_pool(name="qTp", bufs=1)
        qT = qTp.tile([P, H, S], bf16, name="qT")

        # own-frame raw hs stays resident (LN source + residual adds)
        hsop = tc.alloc_tile_pool(name="hsop", bufs=1)
        hso = hsop.tile([P, NQT, D], f32, name="hso")

        # prefetch attn1 weights as [P, ND, D] fp8
        wkvp = tc.alloc_tile_pool(name="wkvp", bufs=1)
        wk = wkvp.tile([P, ND, D], f8, name="wkt", tag="wkt")
        nc.sync.dma_start(out=wk[:],
                          in_=wk1_d.rearrange("(a p) d -> p a d", p=P))
        wv = wkvp.tile([P, ND, D], f8, name="wvt", tag="wvt")
        nc.sync.dma_start(out=wv[:],
                          in_=wv1_d.rearrange("(a p) d -> p a d", p=P))
        wq = wkvp.tile([P, ND, D], f8, name="wqt", tag="wqt")
        nc.sync.dma_start(out=wq[:],
                          in_=wq1_d.rearrange("(a p) d -> p a d", p=P))

        # ---- LN of kv frames (rows 1024:3072 of hs3) -> xkvT ----
        xkvp = tc.alloc_tile_pool(name="xkvp", bufs=1)
        xkvT = xkvp.tile([P, ND, KV], f8, name="xkvT")
        xowp = tc.alloc_tile_pool(name="xowp", bufs=1)
        xowT = xowp.tile([P, ND, S], f8, name="xowT")

        # encoder K/V built here: the LN phase is Vector-bound, PE has slack
        wk2p = tc.alloc_tile_pool(name="wk2p", bufs=1)
        wk2 = wk2p.tile([P, NE, D], f8, name="wk2t", tag="wk2t")
        nc.sync.dma_start(out=wk2[:],
                          in_=wk2_d.rearrange("(a p) d -> p a d", p=P))
        wv2 = wk2p.tile([P, NE, D], f8, name="wv2t", tag="wv2t")
        nc.sync.dma_start(out=wv2[:],
                          in_=wv2_d.rearrange("(a p) d -> p a d", p=P))
        encTp = tc.alloc_tile_pool(name="encTp", bufs=1)
        encT = encTp.tile([P, NE, ENCP], f8, name="encT")
        nc.vector.memset(encT[:], 0.0)
        encp = tc.alloc_tile_pool(name="encp", bufs=2)
        enc_sb = encp.tile([ENC, CROSS], f32, name="enc_sb")
        nc.sync.dma_start(out=enc_sb[:], in_=enc_d)
        enc_bf = encp.tile([ENC, CROSS], bf16, name="enc_bf")
        nc.vector.tensor_copy(enc_bf[:], enc_sb[:])

        ptp = tc.alloc_tile_pool(name="ptp", bufs=3, space=PSUM)
        pk2p = tc.alloc_tile_pool(name="pk2p", bufs=2, space=PSUM)
        pv2p = tc.alloc_tile_pool(name="pv2p", bufs=2, space=PSUM)
        for kt in range(NE):
            ptile = ptp.tile([P, P], bf16, name="ptile", tag="pt")
            nc.tensor.transpose(ptile[0:P, 0:ENC],
                                enc_bf[:, kt * P:(kt + 1) * P],
                                identity[0:ENC, 0:ENC])
            nc.vector.tensor_copy(encT[:, kt, 0:ENC], ptile[0:P, 0:ENC])
        for h in range(H):
            pk2 = pk2p.tile([P, ENCP], f32, name="pk2", tag="pk2")
            for j in range(NE // 2):
                nc.tensor.matmul(pk2[0:DH, 0:ENCP],
                                 wk2[:, 2 * j:2 * j + 2,
                                     h * DH:(h + 1) * DH],
                                 encT[:, 2 * j:2 * j + 2, :],
                                 start=(j == 0), stop=(j == NE // 2 - 1),
                                 perf_mode=DR)
            nc.scalar.copy(out=k2T[0:DH, h, :], in_=pk2[0:DH, :])
        pv20 = pv2p.tile([P, 320], f32, name="pv20", tag="pv2")
        pv21 = pv2p.tile([P, 320], f32, name="pv21", tag="pv2")
        for j in range(NE // 2):
            nc.tensor.matmul(pv20[0:ENC, :],
                             encT[:, 2 * j:2 * j + 2, 0:ENC],
                             wv2[:, 2 * j:2 * j + 2, 0:320],
                             start=(j == 0), stop=(j == NE // 2 - 1),
                             perf_mode=DR)
            nc.tensor.matmul(pv21[0:ENC, :],
                             encT[:, 2 * j:2 * j + 2, 0:ENC],
                             wv2[:, 2 * j:2 * j + 2, 320:640],
                             start=(j == 0), stop=(j == NE // 2 - 1),
                             perf_mode=DR)
        nc.vector.tensor_copy(
            v2p[:, 0:4, 0:DH],
            pv20[0:ENC, :].rearrange("p (a b) -> p a b", b=DH))
        nc.vector.tensor_copy(
            v2p[:, 4:8, 0:DH],
            pv21[0:ENC, :].rearrange("p (a b) -> p a b", b=DH))
        nc.vector.memset(v2p[:, :, DH:DH + 1], 1.0)

        for t in range(NKT):
            xt = io.tile([P, D], f32, name="xt", tag="io")
            nc.sync.dma_start(out=xt[:],
                              in_=hs3_d[(NQT + t) * P:(NQT + t + 1) * P, :])
            ln_block(xt[:], lambda kt, t=t: xkvT[:, kt, t * P:(t + 1) * P],
                     ptp, io)
        # ---- LN of own frame -> xowT ----
        nc.sync.dma_start(out=hso[:],
                          in_=hs3_d[0:S, :].rearrange("(t p) d -> p t d", p=P))
        for t in range(NQT):
            ln_block(hso[:, t, :],
                     lambda kt, t=t: xowT[:, kt, t * P:(t + 1) * P], ptp, io)
        pv2p.release()
        pk2p.release()
        ptp.release()
        encp.release()
        encTp.release()
        wk2p.release()

        # ---- projections ----
        pjp = tc.alloc_tile_pool(name="pjp", bufs=3, space=PSUM)
        pvp = tc.alloc_tile_pool(name="pvp", bufs=2, space=PSUM)

        # K: kT[dh, h, kv]
        for h in range(H):
            for c2 in range(2):
                pk = pjp.tile([P, S], f32, name="pk", tag="pj")
                for j in range(2):
                    for half in range(2):
                        nc.tensor.matmul(
                            pk[0:DH, half * 512:(half + 1) * 512],
                            wk[:, 2 * j:2 * j + 2, h * DH:(h + 1) * DH],
                            xkvT[:, 2 * j:2 * j + 2,
                                 c2 * S + half * 512:c2 * S + (half + 1) * 512],
                            start=(j == 0), stop=False, perf_mode=DR)
                for half in range(2):
                    nc.tensor.matmul(
                        pk[0:DH, half * 512:(half + 1) * 512],
                        wk[:, ND - 1, h * DH:(h + 1) * DH],
                        xkvT[:, ND - 1, c2 * S + half * 512:
                             c2 * S + (half + 1) * 512],
                        start=False, stop=(half == 1))
                if has_kb1:
                    nc.vector.tensor_scalar_add(
                        pk[0:DH, :], pk[0:DH, :],
                        bias_tiles["kb1"][:, h:h + 1])
                if (h + c2) % 2 == 0:
                    nc.scalar.copy(out=kT[0:DH, h, c2 * S:(c2 + 1) * S],
                                   in_=pk[0:DH, :])
                else:
                    nc.vector.tensor_copy(kT[0:DH, h, c2 * S:(c2 + 1) * S],
                                          pk[0:DH, :])

        # V: vp[kv, kvt, h, 0:80] + ones col at 80
        for m in range(NKT):
            pv0 = pvp.tile([P, 320], f32, name="pv0", tag="pv")
            pv1 = pvp.tile([P, 320], f32, name="pv1", tag="pv")
            for j in range(2):
                nc.tensor.matmul(pv0[:],
                                 xkvT[:, 2 * j:2 * j + 2, m * P:(m + 1) * P],
                                 wv[:, 2 * j:2 * j + 2, 0:320],
                                 start=(j == 0), stop=False, perf_mode=DR)
                nc.tensor.matmul(pv1[:],
                                 xkvT[:, 2 * j:2 * j + 2, m * P:(m + 1) * P],
                                 wv[:, 2 * j:2 * j + 2, 320:640],
                                 start=(j == 0), stop=False, perf_mode=DR)
            nc.tensor.matmul(pv0[:], xkvT[:, ND - 1, m * P:(m + 1) * P],
                             wv[:, ND - 1, 0:320], start=False, stop=True)
            nc.tensor.matmul(pv1[:], xkvT[:, ND - 1, m * P:(m + 1) * P],
                             wv[:, ND - 1, 320:640], start=False, stop=True)
            vsl = vp[:, m, :, :]
            nc.vector.tensor_copy(
                vsl[:, 0:4, 0:DH], pv0[:].rearrange("p (a b) -> p a b", b=DH))
            nc.vector.tensor_copy(
                vsl[:, 4:8, 0:DH], pv1[:].rearrange("p (a b) -> p a b", b=DH))
            nc.vector.memset(vsl[:, :, DH:DH + 1], 1.0)

        # Q: qT[dh, h, tok]
        for h in range(H):
            pq = pjp.tile([P, S], f32, name="pq", tag="pj")
            for j in range(2):
                for c in range(2):
                    nc.tensor.matmul(
                        pq[0:DH, c * 512:(c + 1) * 512],
                        wq[:, 2 * j:2 * j + 2, h * DH:(h + 1) * DH],
                        xowT[:, 2 * j:2 * j + 2, c * 512:(c + 1) * 512],
                        start=(j == 0), stop=False, perf_mode=DR)
            for c in range(2):
                nc.tensor.matmul(
                    pq[0:DH, c * 512:(c + 1) * 512],
                    wq[:, ND - 1, h * DH:(h + 1) * DH],
                    xowT[:, ND - 1, c * 512:(c + 1) * 512],
                    start=False, stop=(c == 1))
            if has_qb1:
                nc.vector.tensor_scalar_add(
                    pq[0:DH, :], pq[0:DH, :], bias_tiles["qb1"][:, h:h + 1])
            if h % 2 == 0:
                nc.scalar.copy(out=qT[0:DH, h, :], in_=pq[0:DH, :])
            else:
                nc.vector.tensor_copy(qT[0:DH, h, :], pq[0:DH, :])
        pvp.release()
        pjp.release()
        xowp.release()
        xkvp.release()
        wkvp.release()

        # ================= attn1: attention =================
        o1pp = tc.alloc_tile_pool(name="o1pp", bufs=1)
        o1pt = o1pp.tile([P, H, D], bf16, name="o1pt")
        nc.sync.dma_start(out=o1pt[0:DH, :, :],
                          in_=o1p_d.rearrange("h p d -> p h d"))
        sbavp = tc.alloc_tile_pool(name="sbavp", bufs=1)
        sbavs = []
        for h in range(H):
            sbavs.append(sbavp.tile([DH + 1, S], bf16, name=f"sbav{h}",
                                    tag=f"sbav{h}"))
        denp = tc.alloc_tile_pool(name="denp", bufs=4)
        expp = tc.alloc_tile_pool(name="expp", bufs=3)
        psp = tc.alloc_tile_pool(name="psp", bufs=2, space=PSUM)
        pavp = tc.alloc_tile_pool(name="pavp", bufs=1, space=PSUM)
        pbp = tc.alloc_tile_pool(name="pbp", bufs=1, space=PSUM)

        for h in range(H):
            pav = pavp.tile([P, S], f32, name="pav", tag="pav")
            for kvt in range(NKT):
                ps = psp.tile([P, S], f32, name="ps", tag="ps")
                for half in range(2):
                    nc.tensor.matmul(
                        ps[:, half * 512:(half + 1) * 512],
                        kT[0:DH, h, kvt * P:(kvt + 1) * P],
                        qT[0:DH, h, half * 512:(half + 1) * 512],
                        start=True, stop=True)
                ex = expp.tile([P, S], bf16, name="ex", tag="exp")
                nc.scalar.activation(ex[:], ps[:], AF.Exp, scale=ESCALE)
                for half in range(2):
                    nc.tensor.matmul(
                        pav[0:DH + 1, half * 512:(half + 1) * 512],
                        vp[:, kvt, h, 0:DH + 1],
                        ex[:, half * 512:(half + 1) * 512],
                        start=(kvt == 0), stop=(kvt == NKT - 1))
            nc.vector.tensor_copy(sbavs[h][:], pav[0:DH + 1, :])
            den = denp.tile([1, S], bf16, name="den", tag="den")
            nc.sync.dma_start(out=den[:], in_=sbavs[h][DH:DH + 1, :])
            pb = pbp.tile([P, S], f32, name="pb", tag="pb")
            for c in range(2):
                nc.tensor.matmul(pb[0:DH, c * 512:(c + 1) * 512], onesb[:],
                                 den[:, c * 512:(c + 1) * 512],
                                 start=True, stop=True)
            nc.vector.reciprocal_approx_fast(pb[0:DH, :], pb[0:DH, :])
            nc.vector.tensor_mul(sbavs[h][0:DH, :], sbavs[h][0:DH, :],
                                 pb[0:DH, :])
            if has_vb1:
                nc.vector.tensor_scalar_add(
                    sbavs[h][0:DH, :], sbavs[h][0:DH, :],
                    bias_tiles["vb1"][:, h:h + 1])
        pbp.release()
        pavp.release()
        psp.release()
        expp.release()
        denp.release()

        # o1 projection + residual
        pop = tc.alloc_tile_pool(name="pop", bufs=6, space=PSUM)
        for t in range(NQT):
            po0 = pop.tile([P, 320], f32, name="po0", tag="po")
            po1 = pop.tile([P, 320], f32, name="po1", tag="po")
            for h in range(H):
                nc.tensor.matmul(po0[:], sbavs[h][0:DH, t * P:(t + 1) * P],
                                 o1pt[0:DH, h, 0:320],
                                 start=(h == 0), stop=(h == H - 1))
                nc.tensor.matmul(po1[:], sbavs[h][0:DH, t * P:(t + 1) * P],
                                 o1pt[0:DH, h, 320:640],
                                 start=(h == 0), stop=(h == H - 1))
            nc.vector.tensor_add(h2[:, t, 0:320], po0[:], hso[:, t, 0:320])
            nc.vector.tensor_add(h2[:, t, 320:640], po1[:],
                                 hso[:, t, 320:640])
            if has_ob1:
                nc.vector.tensor_add(h2[:, t, :], h2[:, t, :],
                                     bias_tiles["ob1"][:])
        pop.release()
        sbavp.release()
        o1pp.release()
        hsop.release()
        qTp.release()
        vpp.release()
        kTp.release()
        h3p = tc.alloc_tile_pool(name="h3p", bufs=1)
        h3 = h3p.tile([P, NQT, D], f32, name="h3")

        # ================= attn2: cross attention =================
        q2Tp = tc.alloc_tile_pool(name="q2Tp", bufs=1)
        q2T = q2Tp.tile([P, H, S], bf16, name="q2T")
        x2p = tc.alloc_tile_pool(name="x2p", bufs=1)
        x2T = x2p.tile([P, ND, S], f8, name="x2T")
        sbav2p = tc.alloc_tile_pool(name="sbav2p", bufs=1)
        den2rp = tc.alloc_tile_pool(name="den2rp", bufs=1)
        dens2 = []
        sbavs2 = []
        for h in range(H):
            dens2.append(den2rp.tile([1, S], bf16, name=f"d2en{h}",
                                     tag=f"d2en{h}"))
            sbavs2.append(sbav2p.tile([DH + 1, S], bf16, name=f"sbav2{h}",
                                      tag=f"sbav2{h}"))
        pjp2 = tc.alloc_tile_pool(name="pjp2", bufs=2, space=PSUM)

        wq2p = tc.alloc_tile_pool(name="wq2p", bufs=1)
        wq2 = wq2p.tile([P, ND, D], f8, name="wq2t", tag="wq2t")
        nc.sync.dma_start(out=wq2[:],
                          in_=wq2_d.rearrange("(a p) d -> p a d", p=P))

        ptp2 = tc.alloc_tile_pool(name="ptp2", bufs=3, space=PSUM)
        for t in range(NQT):
            ln_block(h2[:, t, :],
                     lambda kt, t=t: x2T[:, kt, t * P:(t + 1) * P], ptp2, io)
        ptp2.release()

        # q2 projection
        for h in range(H):
            pq = pjp2.tile([P, S], f32, name="pq2", tag="pj2")
            for j in range(2):
                for c in range(2):
                    nc.tensor.matmul(
                        pq[0:DH, c * 512:(c + 1) * 512],
                        wq2[:, 2 * j:2 * j + 2, h * DH:(h + 1) * DH],
                        x2T[:, 2 * j:2 * j + 2, c * 512:(c + 1) * 512],
                        start=(j == 0), stop=False, perf_mode=DR)
            for c in range(2):
                nc.tensor.matmul(
                    pq[0:DH, c * 512:(c + 1) * 512],
                    wq2[:, ND - 1, h * DH:(h + 1) * DH],
                    x2T[:, ND - 1, c * 512:(c + 1) * 512],
                    start=False, stop=(c == 1))
            if has_q2b:
                nc.vector.tensor_scalar_add(
                    pq[0:DH, :], pq[0:DH, :], bias_tiles["q2b"][:, h:h + 1])
            if h % 2 == 0:
                nc.scalar.copy(out=q2T[0:DH, h, :], in_=pq[0:DH, :])
            else:
                nc.vector.tensor_copy(q2T[0:DH, h, :], pq[0:DH, :])

        wq2p.release()

        # attention 2
        exp2p = tc.alloc_tile_pool(name="exp2p", bufs=3)
        pav2p = tc.alloc_tile_pool(name="pav2p", bufs=2, space=PSUM)
        for h in range(H):
            ps2 = pjp2.tile([P, S], f32, name="ps2", tag="pj2")
            for c in range(2):
                nc.tensor.matmul(ps2[0:ENC, c * 512:(c + 1) * 512],
                                 k2T[0:DH, h, 0:ENC],
                                 q2T[0:DH, h, c * 512:(c + 1) * 512],
                                 start=True, stop=True)
            ex2 = exp2p.tile([P, S], bf16, name="ex2", tag="exp2")
            nc.scalar.activation(ex2[0:ENC, :], ps2[0:ENC, :], AF.Exp,
                                 scale=ESCALE)
            pav2 = pav2p.tile([P, S], f32, name="pav2", tag="pav2")
            for c in range(2):
                nc.tensor.matmul(pav2[0:DH + 1, c * 512:(c + 1) * 512],
                                 v2p[:, h, 0:DH + 1],
                                 ex2[0:ENC, c * 512:(c + 1) * 512],
                                 start=True, stop=True)
            nc.vector.tensor_copy(sbavs2[h][:], pav2[0:DH + 1, :])
            nc.sync.dma_start(out=dens2[h][:], in_=sbavs2[h][DH:DH + 1, :])
        pav2p.release()
        exp2p.release()

        # normalize + o2 projection + residual -> h3
        o2pp = tc.alloc_tile_pool(name="o2pp", bufs=1)
        o2pt = o2pp.tile([P, H, D], bf16, name="o2pt")
        nc.sync.dma_start(out=o2pt[0:DH, :, :],
                          in_=o2p_d.rearrange("h p d -> p h d"))
        pop2 = tc.alloc_tile_pool(name="pop2", bufs=4, space=PSUM)
        for h in range(H):
            pb = pjp2.tile([P, S], f32, name="pb2", tag="pj2")
            for c in range(2):
                nc.tensor.matmul(pb[0:DH, c * 512:(c + 1) * 512], onesb[:],
                                 dens2[h][:, c * 512:(c + 1) * 512],
                                 start=True, stop=True)
            nc.vector.reciprocal_approx_fast(pb[0:DH, :], pb[0:DH, :])
            nc.vector.tensor_mul(sbavs2[h][0:DH, :], sbavs2[h][0:DH, :],
                                 pb[0:DH, :])
        for t in range(NQT):
            po0 = pop2.tile([P, 320], f32, name="po20", tag="po2")
            po1 = pop2.tile([P, 320], f32, name="po21", tag="po2")
            for h in range(H):
                nc.tensor.matmul(po0[:], sbavs2[h][0:DH, t * P:(t + 1) * P],
                                 o2pt[0:DH, h, 0:320],
                                 start=(h == 0), stop=(h == H - 1))
                nc.tensor.matmul(po1[:], sbavs2[h][0:DH, t * P:(t + 1) * P],
                                 o2pt[0:DH, h, 320:640],
                                 start=(h == 0), stop=(h == H - 1))
            nc.vector.tensor_add(h3[:, t, 0:320], po0[:], h2[:, t, 0:320])
            nc.vector.tensor_add(h3[:, t, 320:640], po1[:], h2[:, t, 320:640])
            if has_ob2:
                nc.vector.tensor_add(h3[:, t, :], h3[:, t, :],
                                     bias_tiles["ob2"][:])
        pop2.release()
        o2pp.release()
        pjp2.release()
        den2rp.release()
        sbav2p.release()
        x2p.release()
        q2Tp.release()

        # ================= FFN (geglu) =================
        hgTp = tc.alloc_tile_pool(name="hgTp", bufs=1)
        hgT = hgTp.tile([P, NK2, S], f8, name="hgT")
        x3p = tc.alloc_tile_pool(name="x3p", bufs=1)
        x3T = x3p.tile([P, ND, S], f8, name="x3T")
        ptp3 = tc.alloc_tile_pool(name="ptp3", bufs=3, space=PSUM)
        for t in range(NQT):
            ln_block(h3[:, t, :],
                     lambda kt, t=t: x3T[:, kt, t * P:(t + 1) * P], ptp3, io)
        ptp3.release()

        w1pp = tc.alloc_tile_pool(name="w1pp", bufs=6)
        ggp = tc.alloc_tile_pool(name="ggp", bufs=3)
        pw1 = tc.alloc_tile_pool(name="pw1", bufs=4, space=PSUM)
        for mp in range(NK2):
            wh = w1pp.tile([P, ND, P], f8, name="wh", tag="w1")
            nc.sync.dma_start(out=wh[:], in_=w1p_d[mp])
            wg = w1pp.tile([P, ND, P], f8, name="wg", tag="w1")
            nc.sync.dma_start(out=wg[:], in_=w1p_d[mp + NK2])
            ph = pw1.tile([P, S], f32, name="ph", tag="pw1")
            pg = pw1.tile([P, S], f32, name="pg", tag="pw1")
            for j in range(2):
                for c in range(2):
                    nc.tensor.matmul(ph[:, c * 512:(c + 1) * 512],
                                     wh[:, 2 * j:2 * j + 2, :],
                                     x3T[:, 2 * j:2 * j + 2,
                                         c * 512:(c + 1) * 512],
                                     start=(j == 0), stop=False, perf_mode=DR)
            for c in range(2):
                nc.tensor.matmul(ph[:, c * 512:(c + 1) * 512],
                                 wh[:, ND - 1, :],
                                 x3T[:, ND - 1, c * 512:(c + 1) * 512],
                                 start=False, stop=(c == 1))
            for j in range(2):
                for c in range(2):
                    nc.tensor.matmul(pg[:, c * 512:(c + 1) * 512],
                                     wg[:, 2 * j:2 * j + 2, :],
                                     x3T[:, 2 * j:2 * j + 2,
                                         c * 512:(c + 1) * 512],
                                     start=(j == 0), stop=False, perf_mode=DR)
            for c in range(2):
                nc.tensor.matmul(pg[:, c * 512:(c + 1) * 512],
                                 wg[:, ND - 1, :],
                                 x3T[:, ND - 1, c * 512:(c + 1) * 512],
                                 start=False, stop=(c == 1))
            gg = ggp.tile([P, S], bf16, name="gg", tag="gg")
            nc.scalar.activation(gg[:], pg[:], AF.Gelu_apprx_tanh,
                                 bias=b1pt[:, mp + NK2:mp + NK2 + 1],
                                 scale=1.0 / SW1)
            nc.vector.scalar_tensor_tensor(
                out=hgT[:, mp, :],
                in0=ph[:], scalar=b1pt[:, mp:mp + 1], in1=gg[:],
                op0=Alu.add, op1=Alu.mult)
        pw1.release()
        ggp.release()
        w1pp.release()
        x3p.release()

        w2pp = tc.alloc_tile_pool(name="w2pp", bufs=6)
        pw2 = tc.alloc_tile_pool(name="pw2", bufs=8, space=PSUM)
        for th in range(2):
            pf = []
            for tt in range(4):
                pf.append((pw2.tile([P, 320], f32, name=f"pf{tt}a", tag="pw2"),
                           pw2.tile([P, 320], f32, name=f"pf{tt}b", tag="pw2")))
            for kj in range(NK2 // 2):
                w2t = w2pp.tile([P, 2, D], f8, name="w2t", tag="w2")
                nc.sync.dma_start(
                    out=w2t[:],
                    in_=w2p_d[2 * kj:2 * kj + 2]
                    .rearrange("a p d -> p a d"))
                for tt in range(4):
                    t = th * 4 + tt
                    nc.tensor.matmul(pf[tt][0][:],
                                     hgT[:, 2 * kj:2 * kj + 2,
                                         t * P:(t + 1) * P],
                                     w2t[:, :, 0:320],
                                     start=(kj == 0), stop=(kj == NK2 // 2 - 1),
                                     perf_mode=DR)
                    nc.tensor.matmul(pf[tt][1][:],
                                     hgT[:, 2 * kj:2 * kj + 2,
                                         t * P:(t + 1) * P],
                                     w2t[:, :, 320:640],
                                     start=(kj == 0), stop=(kj == NK2 // 2 - 1),
                                     perf_mode=DR)
            for tt in range(4):
                t = th * 4 + tt
                ot = io.tile([P, D], f32, name="ot", tag="io")
                nc.vector.scalar_tensor_tensor(
                    out=ot[:, 0:320], in0=pf[tt][0][:], scalar=ODESC,
                    in1=h3[:, t, 0:320], op0=Alu.mult, op1=Alu.add)
                nc.vector.scalar_tensor_tensor(
                    out=ot[:, 320:640], in0=pf[tt][1][:], scalar=ODESC,
                    in1=h3[:, t, 320:640], op0=Alu.mult, op1=Alu.add)
                if has_fb2:
                    nc.vector.tensor_add(ot[:], ot[:], bias_tiles["fb2"][:])
                nc.sync.dma_start(out=out_d[t * P:(t + 1) * P, :], in_=ot[:])
        pw2.release()
        w2pp.release()
        hgTp.release()

        h3p.release()
        v2pp.release()
        k2Tp.release()
        h2p.release()
        io.release()
        stats.release()
        const.release()

    nc.compile()
    return nc


def _prep_inputs(inputs):
    import ml_dtypes

    f32 = np.float32
    bf16 = ml_dtypes.bfloat16
    f8 = ml_dtypes.float8_e4m3
    g = {k: np.asarray(v) for k, v in inputs.items()}
    hs = np.ascontiguousarray(g["hidden_states"], f32)
    enc = np.ascontiguousarray(g["encoder_hidden_states"], f32)
    f = int(g["video_length"])
    assert hs.shape == (F, S, D) and enc.shape == (F, ENC, CROSS) and f == F

    ln1w, ln1b = g["ln1_w"].astype(f32), g["ln1_b"].astype(f32)
    ln2w, ln2b = g["ln2_w"].astype(f32), g["ln2_b"].astype(f32)
    ln3w, ln3b = g["ln3_w"].astype(f32), g["ln3_b"].astype(f32)
    q1, k1, v1 = (g[n].astype(f32) for n in ("q1", "k1", "v1"))
    o1w, o1b = g["o1_w"].astype(f32), g["o1_b"].astype(f32)
    q2, k2, v2 = (g[n].astype(f32) for n in ("q2", "k2", "v2"))
    o2w, o2b = g["o2_w"].astype(f32), g["o2_b"].astype(f32)
    w1, b1 = g["ff_w1"].astype(f32), g["ff_b1"].astype(f32)
    w2, b2 = g["ff_w2"].astype(f32), g["ff_b2"].astype(f32)

    b1f = (b1 + ln3b @ w1).reshape(NM1, P).T.copy()
    b1f[:, 0:NK2] *= SW1
    shared = {
        "wq1": np.ascontiguousarray(q1 * ln1w[:, None] * SW).astype(f8),
        "wk1": np.ascontiguousarray(k1 * ln1w[:, None] * SW).astype(f8),
        "wv1": np.ascontiguousarray(v1 * ln1w[:, None] * SW).astype(f8),
        "o1p": np.ascontiguousarray(o1w.reshape(H, DH, D) / SW).astype(bf16),
        "wq2": np.ascontiguousarray(q2 * ln2w[:, None] * SW).astype(f8),
        "wk2": np.ascontiguousarray(k2 * SW).astype(f8),
        "wv2": np.ascontiguousarray(v2 * SW).astype(f8),
        "o2p": np.ascontiguousarray(o2w.reshape(H, DH, D) / SW).astype(bf16),
        "w1p": np.ascontiguousarray(
            (w1 * ln3w[:, None] * SW1).reshape(ND, P, NM1, P)
            .transpose(2, 1, 0, 3)).astype(f8),
        "b1p": np.ascontiguousarray(b1f),
        "w2p": np.ascontiguousarray(w2.reshape(NK2, P, D) * SW2).astype(f8),
    }

    qb1 = (ln1b @ q1) * SW
    kb1 = (ln1b @ k1) * SW
    vb1 = (ln1b @ v1) * SW
    q2b = (ln2b @ q2) * SW
    flags = (
        bool(np.any(qb1)), bool(np.any(kb1)), bool(np.any(vb1)),
        bool(np.any(o1b)), bool(np.any(q2b)), bool(np.any(o2b)),
        bool(np.any(b2)),
    )
    has_qb1, has_kb1, has_vb1, has_ob1, has_q2b, has_ob2, has_fb2 = flags
    if has_qb1:
        shared["qb1"] = np.ascontiguousarray(qb1.reshape(H, DH).T)
    if has_kb1:
        shared["kb1"] = np.ascontiguousarray(kb1.reshape(H, DH).T)
    if has_vb1:
        shared["vb1"] = np.ascontiguousarray(vb1.reshape(H, DH).T)
    if has_q2b:
        shared["q2b"] = np.ascontiguousarray(q2b.reshape(H, DH).T)
    if has_ob1:
        shared["ob1"] = np.ascontiguousarray(np.broadcast_to(o1b, (P, D)))
    if has_ob2:
        shared["ob2"] = np.ascontiguousarray(np.broadcast_to(o2b, (P, D)))
    if has_fb2:
        shared["fb2"] = np.ascontiguousarray(np.broadcast_to(b2, (P, D)))

    former = [0] + list(range(F - 1))
    in_maps = []
    for i in range(F):
        m = dict(shared)
        m["hs3"] = np.ascontiguousarray(
            np.concatenate([hs[i], hs[0], hs[former[i]]], axis=0))
        m["enc"] = np.ascontiguousarray(enc[i])
        in_maps.append(m)
    return flags, in_maps


def get_program(flags):
    if flags not in _PROGRAM_CACHE:
        _PROGRAM_CACHE[flags] = _build_program(flags)
    return _PROGRAM_CACHE[flags]


def run(inputs, trace=False):
    from concourse.bass_utils import run_bass_kernel_spmd

    flags, in_maps = _prep_inputs(inputs)
    nc = get_program(flags)
    res = run_bass_kernel_spmd(nc, in_maps, core_ids=list(range(F)),
                               trace=trace)
    out = np.stack([r["out"] for r in res.results], axis=0)
    return out.astype(np.float32), res


def kernel(**inputs):
    out, _ = run(inputs, trace=False)
    return out
